# revision 1
# baseline (speedup 1.0000x reference)
"""Trainium2 Bass kernel for nn_DGM_77318001263213 (dense_transformer).

Reference computation (per batch b of 16):
  dir_map = conv3x3_SAME(x, dc_w) + dc_b            [12, 64, 64]
  q = conv2x2_s2(dir_map, q_w) + q_b  -> [48, 1024]
  k = conv2x2_s2(dir_map, k_w) + k_b  -> [48, 1024]
  v = conv2x2_s2(x, v_w) + v_b        -> [1024, 1024]
  attn = softmax(q^T k, axis=-1)                    [1024, 1024]
  out[c, m] = sum_n v[c, n] * attn[m, n]            [1024, 1024]

Device mapping (data-parallel, 2 batches per core on 8 cores):
  * q,k are computed as ONE composite 4x4 stride-2 convolution of x
    (the 3x3 dc conv and 2x2 proj convs are both linear, so they fold
    into a single 4x4 kernel on the host) with 96 output channels
    (q stacked with k) -- keeps PE matmul M=96 instead of M=12.
  * v conv computed transposed: V^T[n, oc] tiles via matmuls with
    lhsT = host-prearranged space-to-depth x (the stationary operand
    must have a single free dim), rhs = reshaped v weights.
  * scores computed transposed: T[n, m] = S[m, n] = sum_c k[c,n] q[c,m],
    then E = exp(T) (no max subtraction; |S| <= ~25 so exp is safe in
    f32), unnormalized U^T[m, c] = sum_n E[n, m] V^T[n, c], row sums
    D[m] = sum_n E[n, m] via ones-matmul, and out^T[m, c] =
    U^T[m, c] / D[m] applied as a per-partition scale on eviction.
  * all matmuls run in float32r (fp32 data, ~1.6e-4 matmul rel err,
    4x faster than fp32 on the PE).
  * host transposes out^T -> out at gather time.

Measured on trn2 (8 cores, axon): HW exec ~213-250 us per core pair of
batches (run-to-run spread is chip power-state/clock variance), output
rel absmax err vs the fp32 reference ~9.3e-4 (fp32r matmul rounding).
"""
import os
import sys
import types
import numpy as np
from contextlib import ExitStack

for _p in ("/opt/trn_rl_repo", "/root/.axon_site/_ro/trn_rl_repo"):
    if os.path.isdir(_p) and _p not in sys.path:
        sys.path.insert(0, _p)

import concourse.bacc as bacc
import concourse.bass as bass
import concourse.tile as tile
import concourse.mybir as mybir
from concourse import bass_utils

F32 = mybir.dt.float32
F32R = mybir.dt.float32r
ts = bass.ts

NCORES = 8
BPC = 2          # batches per core
C_IN = 256
HP = 66          # padded spatial
NPOS = 1024      # 32*32 output positions
OC = 1024        # v output channels


def _install_ntff_hook_shim():
    """Register the axon NTFF profile hook if the image's antenv lacks it.

    Only needed when BASS_TRACE=1; harmless otherwise."""
    if "antenv.axon_hooks" in sys.modules:
        return
    try:
        from trn_agent_boot.trn_boot import _ntff_profile_via_ctypes
        hook = _ntff_profile_via_ctypes("/opt/axon/libaxon_pjrt.so")
    except Exception:
        hook = None
    m = types.ModuleType("antenv.axon_hooks")
    m.get_axon_ntff_profile_hook = lambda: hook
    m.set_axon_ntff_profile_hook = lambda h: None
    sys.modules["antenv.axon_hooks"] = m


def build_program():
    """Build the per-core Bacc program (same program on all 8 cores)."""
    nc = bacc.Bacc(trn_type="TRN2", target_bir_lowering=False, debug=False)

    # padded x as 4 stride-2 parity planes: xq[b, c, a*2+p, r, s] =
    # x_pad[b, c, 2r+a, 2s+p] -- makes every q/k-conv tap view have a
    # stride-1 innermost free dim (stride-2 reads halve PE stream rate)
    xq = nc.dram_tensor("xq", [BPC, C_IN, 4, 33, 33], F32, kind="ExternalInput")
    # wqk chunk order = (h, pl, u, v) == the exact consumption order of the
    # q/k conv loop, so the per-group DMAs stream just ahead of the matmuls
    wqk = nc.dram_tensor("wqk", [128, 32, 96], F32, kind="ExternalInput")
    wv = nc.dram_tensor("wv", [128, 8, 1024], F32, kind="ExternalInput")
    bqk = nc.dram_tensor("bqk", [96, 1], F32, kind="ExternalInput")
    bvr = nc.dram_tensor("bvr", [128, 1024], F32, kind="ExternalInput")
    o = nc.dram_tensor("o", [BPC, 1024, 1024], F32, kind="ExternalOutput")

    EXP = mybir.ActivationFunctionType.Exp
    COPY = mybir.ActivationFunctionType.Copy

    with tile.TileContext(nc) as tc, ExitStack() as ctx:
        const = ctx.enter_context(tc.tile_pool(name="const", bufs=1))
        xpool = ctx.enter_context(tc.tile_pool(name="xpool", bufs=8))
        xspool = ctx.enter_context(tc.tile_pool(name="xspool", bufs=8))
        qkp = ctx.enter_context(tc.tile_pool(name="qkp", bufs=1))
        epool = ctx.enter_context(tc.tile_pool(name="epool", bufs=1))
        vtpool = ctx.enter_context(tc.tile_pool(name="vtpool", bufs=1))
        outp = ctx.enter_context(tc.tile_pool(name="outp", bufs=4))
        misc = ctx.enter_context(tc.tile_pool(name="misc", bufs=2))
        ppq = ctx.enter_context(tc.tile_pool(name="ppq", bufs=1, space="PSUM"))
        ppt = ctx.enter_context(tc.tile_pool(name="ppt", bufs=2, space="PSUM"))
        ppv = ctx.enter_context(tc.tile_pool(name="ppv", bufs=2, space="PSUM"))
        ppd = ctx.enter_context(tc.tile_pool(name="ppd", bufs=1, space="PSUM"))
        ppu = ctx.enter_context(tc.tile_pool(name="ppu", bufs=2, space="PSUM"))

        # ---- persistent constants ----
        # wqk in 8 per-(half, plane) group tiles, DMA'd in consumption order
        wqk_g = []
        for g in range(8):
            wt = const.tile([128, 4, 96], F32R, tag=f"wqk_g{g}")
            nc.sync.dma_start(wt[:], wqk.ap().bitcast(F32R)[:, 4 * g : 4 * g + 4, :])
            wqk_g.append(wt)
        # wv/bvr are only needed by the v conv (mid-pipeline) -> ACT HWDGE
        # queue so they don't delay the latency-critical wqk/xq loads.
        wv_t = []
        for ck in range(8):
            wt = const.tile([128, 1024], F32R, tag=f"wv_sb{ck}")
            nc.scalar.dma_start(wt[:], wv.ap().bitcast(F32R)[:, ck, :])
            wv_t.append(wt)
        bqk_sb = const.tile([96, 1], F32, tag="bqk_sb")
        nc.sync.dma_start(bqk_sb[:], bqk.ap())
        bvr_sb = const.tile([128, 1024], F32, tag="bvr_sb")
        nc.scalar.dma_start(bvr_sb[:], bvr.ap())
        # N=2 ones for the D-sum matmuls: fp32r requires an even innermost
        # moving count, and fp32 matmuls lower to 2 half-rate HW matmuls --
        # f32r at N=2 is the cheapest legal form.  memset can't write f32r,
        # so round through an ACT copy.
        ones_f32 = const.tile([128, 2], F32, tag="ones_f32")
        nc.vector.memset(ones_f32[:], 1.0)
        ones2 = const.tile([128, 2], F32R, tag="ones2")
        nc.scalar.copy(ones2[:], ones_f32[:])

        for b in range(BPC):
            # ---- load parity-plane x: one tile per (half, plane), DMA'd
            #      interleaved with the wqk groups in consumption order ----
            xh = []
            for h in range(2):
                planes = []
                for pl in range(4):
                    xt = xpool.tile([128, 33, 33], F32R, tag="xh")
                    nc.sync.dma_start(
                        xt[:], xq.ap().bitcast(F32R)[b, ts(h, 128), pl]
                    )
                    planes.append(xt)
                xh.append(planes)
            # space-to-depth x for the v conv, derived ON DEVICE from the
            # parity planes (same bytes as the DMA'd xq -- saves 4.2 MB of
            # HBM traffic per batch, which paced the whole first batch).
            # The v-conv stationary operand needs a single flat free dim,
            # hence the copy into a contiguous [128, 1024] tile.
            xs_c = []
            for ck in range(8):
                t, h = divmod(ck, 2)
                dy, dx = divmod(t, 2)
                a, u2 = (dy + 1) % 2, (dy + 1) // 2
                p2, v2 = (dx + 1) % 2, (dx + 1) // 2
                xst = xspool.tile([128, 1024], F32R, tag="xs")
                srcv = xh[h][a * 2 + p2][:, u2 : u2 + 32, v2 : v2 + 32]
                dstv = xst[:].rearrange("p (a b) -> p a b", a=32)
                if ck % 2 == 0:
                    nc.vector.tensor_copy(dstv, srcv)
                else:
                    nc.scalar.copy(dstv, srcv)
                xs_c.append(xst)


            # ---- composite q|k conv: psum[96, 512] per m-chunk ----
            # plane-major tap order so matmuls start as soon as the first
            # plane's DMA lands
            QK = qkp.tile([96, 1024], F32R, tag="QK")
            for jm in range(2):
                pq_t = ppq.tile([96, 512], F32, tag="pq")
                first = True
                for h in range(2):
                    for pl in range(4):
                        for u in range(2):
                            for v in range(2):
                                ck2 = h * 16 + pl * 4 + u * 2 + v
                                rhs = xh[h][pl][
                                    :, u + 16 * jm : u + 16 * jm + 16, v : v + 32
                                ]
                                nc.tensor.matmul(
                                    pq_t[:], wqk_g[ck2 // 4][:, ck2 % 4, :], rhs,
                                    start=first,
                                    stop=(h == 1 and pl == 3 and u == 1 and v == 1),
                                )
                                first = False
                nc.vector.tensor_scalar_add(QK[:, ts(jm, 512)], pq_t[:], bqk_sb[:, :1])
            # move q rows (48:96) to partition base 0 for use as matmul rhs
            Qs = qkp.tile([48, 1024], F32R, tag="Qs")
            nc.sync.dma_start(Qs[:], QK[48:96, :])

            # ---- v conv (V^T tiles) with the transposed-scores matmuls,
            #      exps, and Esum partial sums interleaved so the in-order
            #      PE never sits behind an ACT exp or a psum-bank release ----
            e_sb = epool.tile([128, 8, 1024], F32R, tag="e_sb")
            vt_sb = vtpool.tile([128, 8, 1024], F32R, tag="vt_sb")
            esum = epool.tile([128, 1024], F32R, tag="esum")
            for g in range(16):
                jn, l = divmod(g, 2)
                pv_t = ppv.tile([128, 512], F32, tag="pv")
                for ck in range(8):
                    nc.tensor.matmul(
                        pv_t[:], xs_c[ck][:, ts(jn, 128)], wv_t[ck][:, ts(l, 512)],
                        start=(ck == 0), stop=(ck == 7),
                    )
                nc.vector.tensor_add(
                    vt_sb[:, jn, ts(l, 512)], pv_t[:], bvr_sb[:, ts(l, 512)]
                )
                # scores chunk g: T[n, m] = S[m, n] for n-chunk g//2,
                # m-half g%2 -- a single matmul + exp slotted between the
                # v-conv groups
                sn, sm = divmod(g, 2)
                pt_t = ppt.tile([128, 512], F32, tag="pt")
                nc.tensor.matmul(
                    pt_t[:], QK[0:48, ts(sn, 128)], Qs[:, ts(sm, 512)],
                    start=True, stop=True,
                )
                nc.scalar.activation(e_sb[:, sn, ts(sm, 512)], pt_t[:], EXP)
                # Esum partials: esum[p, m] = sum_jn e_sb[p, jn, m], built
                # as chunks complete (D[m] then needs only one 128-deep
                # matmul per m-chunk instead of 8)
                if g % 2 == 1:
                    if sn == 1:
                        nc.any.tensor_add(
                            esum[:], e_sb[:, 0, :], e_sb[:, 1, :]
                        )
                    elif sn > 1:
                        nc.any.tensor_add(esum[:], esum[:], e_sb[:, sn, :])

            # ---- U^T[m, c] = sum_n E[n, m] V^T[n, c]; D[m]; out^T = U^T/D ----
            for mm in range(8):
                pd_t = ppd.tile([128, 2], F32, tag="pd")
                nc.tensor.matmul(
                    pd_t[:], esum[:, ts(mm, 128)], ones2[:], start=True, stop=True
                )
                rc = misc.tile([128, 1], F32, tag="rc")
                nc.vector.reciprocal(rc[:], pd_t[:, 0:1])
                for l in range(2):
                    pu_t = ppu.tile([128, 512], F32, tag="pu")
                    for jn in range(8):
                        nc.tensor.matmul(
                            pu_t[:], e_sb[:, jn, ts(mm, 128)], vt_sb[:, jn, ts(l, 512)],
                            start=(jn == 0), stop=(jn == 7),
                        )
                    ot = outp.tile([128, 512], F32, tag="ot")
                    nc.scalar.activation(ot[:], pu_t[:], COPY, scale=rc[:])
                    # SWDGE queue: keeps output traffic off the SP queue so
                    # the next batch's x planes aren't stuck behind it
                    nc.gpsimd.dma_start(
                        o.ap()[b, ts(mm, 128), ts(l, 512)], ot[:]
                    )

    nc.compile()
    return nc


def host_weights(dc_w, dc_b, q_w, k_w, q_b, k_b, v_w, v_b):
    """Fold dc conv into q/k projections -> composite 4x4 stride-2 weights."""
    dc_w = np.asarray(dc_w, np.float32)
    dc_b = np.asarray(dc_b, np.float32)
    q_w = np.asarray(q_w, np.float32)
    k_w = np.asarray(k_w, np.float32)
    q_b = np.asarray(q_b, np.float32)
    k_b = np.asarray(k_b, np.float32)
    v_w = np.asarray(v_w, np.float32)
    v_b = np.asarray(v_b, np.float32)

    C = dc_w.shape[1]
    Wq = np.zeros((48, C, 4, 4), np.float64)
    Wk = np.zeros((48, C, 4, 4), np.float64)
    for p in range(2):
        for qq in range(2):
            qw_pq = q_w[:, :, p, qq].astype(np.float64)
            kw_pq = k_w[:, :, p, qq].astype(np.float64)
            for dy in range(3):
                for dx in range(3):
                    dcw_dd = dc_w[:, :, dy, dx].astype(np.float64)
                    Wq[:, :, p + dy, qq + dx] += qw_pq @ dcw_dd
                    Wk[:, :, p + dy, qq + dx] += kw_pq @ dcw_dd
    bq_eff = q_b + q_w.sum(axis=(2, 3)) @ dc_b
    bk_eff = k_b + k_w.sum(axis=(2, 3)) @ dc_b
    # lhsT row index = (A*4+B)*C + c', columns: k 0:48 | q 48:96
    # (the device uses QK[0:48] as the scores lhsT (indexes n -> k) and
    #  QK[48:96] as the scores rhs (indexes m -> q))
    wqk_ab = (
        np.concatenate(
            [
                Wk.transpose(2, 3, 1, 0).reshape(16 * C, 48),
                Wq.transpose(2, 3, 1, 0).reshape(16 * C, 48),
            ],
            axis=1,
        )
        .astype(np.float32)
        .reshape(32, 128, 96)  # chunk_old = (A*4+B)*2 + h
    )
    # permute chunks into device consumption order (h, pl, u, v)
    perm = []
    for h in range(2):
        for pl in range(4):
            a, p = divmod(pl, 2)
            for u in range(2):
                for v in range(2):
                    A, Bo = 2 * u + a, 2 * v + p
                    perm.append((A * 4 + Bo) * 2 + h)
    wqk = wqk_ab[perm].transpose(1, 0, 2)  # [part 128, chunk2 32, 96]
    bqk = np.concatenate([bk_eff, bq_eff]).reshape(96, 1).astype(np.float32)
    # v rhs: row = (dy*2+dx)*C + c', col = oc
    wv = np.ascontiguousarray(
        v_w.transpose(2, 3, 1, 0).reshape(8, 128, 4 * C).transpose(1, 0, 2)
    )  # [part 128, chunk 8, oc]
    bvr = np.ascontiguousarray(np.broadcast_to(v_b, (128, 4 * C)))
    return wqk, bqk, wv, bvr


_PROGRAM = None
LAST_RESULTS = None


def _get_program():
    global _PROGRAM
    if _PROGRAM is None:
        _PROGRAM = build_program()
    return _PROGRAM


def kernel(x, dc_w, dc_b, q_w, q_b, k_w, k_b, v_w, v_b):
    _install_ntff_hook_shim()
    x = np.asarray(x, np.float32)
    B = x.shape[0]
    xp = np.pad(x, ((0, 0), (0, 0), (1, 1), (1, 1)))
    # parity planes: xq[b, c, a*2+p, r, s] = x_pad[b, c, 2r+a, 2s+p]
    xq = (
        xp.reshape(B, C_IN, 33, 2, 33, 2)
        .transpose(0, 1, 3, 5, 2, 4)
        .reshape(B, C_IN, 4, 33, 33)
    )
    wqk, bqk, wv, bvr = host_weights(dc_w, dc_b, q_w, k_w, q_b, k_b, v_w, v_b)

    nc = _get_program()
    in_maps = []
    for c in range(NCORES):
        in_maps.append(
            {
                "xq": np.ascontiguousarray(xq[BPC * c : BPC * (c + 1)]),
                "wqk": wqk,
                "wv": wv,
                "bqk": bqk,
                "bvr": bvr,
            }
        )
    res = bass_utils.run_bass_kernel_spmd(nc, in_maps, core_ids=list(range(NCORES)))
    global LAST_RESULTS
    LAST_RESULTS = res

    out = np.empty((B, 1024, 1024), np.float32)
    for c in range(NCORES):
        out[BPC * c : BPC * (c + 1)] = res.results[c]["o"].transpose(0, 2, 1)
    return out

